# revision 1
# baseline (speedup 1.0000x reference)
"""CrossBlock Trainium2 kernel: 8-way SPMD (batch x token-half sharding).

Each core handles batch b = core//2, token half = core%2 for BOTH streams.
Inputs are passed per-core pre-rolled along tokens so queries are rows 0:1024.
"""

import numpy as np
from contextlib import ExitStack

import concourse.bass as bass
import concourse.tile as tile
import concourse.mybir as mybir
from concourse import bacc
from concourse.bass_utils import run_bass_kernel_spmd

F32 = mybir.dt.float32
F32R = mybir.dt.float32r
BF16 = mybir.dt.bfloat16
AF = mybir.ActivationFunctionType
OP = mybir.AluOpType

N, E, H, DH = 2048, 256, 4, 64
T = N // 2  # queries per core
SCALE = DH ** -0.5


def build_program(use_ln=False, use_bias=False):
    nc = bacc.Bacc("TRN2", target_bir_lowering=False, debug=False, num_devices=8)

    def dri(name, shape, dt=F32):
        return nc.dram_tensor(name, shape, dt, kind="ExternalInput").ap()

    t_ = {}
    t_["x"] = [dri("x0", [N, E], F32R), dri("x1", [N, E], F32R)]
    for nm, sh in (("Wqk", [E, E]), ("Wv", [E, E]), ("Wo", [E, E]),
                   ("W1", [2 * E, 2 * E]), ("W2", [2 * E, E])):
        t_[nm] = dri(nm, sh, F32R)
    if use_bias:
        for nm, w in (("bqk", E), ("bv", E), ("bo", E), ("b1", 2 * E), ("b2", E)):
            t_[nm] = dri(nm, [1, w], F32R)
    if use_ln:
        for nm in ("lng", "lnb"):
            t_[nm] = dri(nm, [1, 2 * E], F32R)
    t_["ident"] = dri("ident", [128, 128], F32R)
    t_["ones"] = dri("ones_in", [128, 512], F32R)
    t_["y"] = [
        nc.dram_tensor("y0", [T, E], F32, kind="ExternalOutput").ap(),
        nc.dram_tensor("y1", [T, E], F32, kind="ExternalOutput").ap(),
    ]

    with tile.TileContext(nc) as tc:
        with ExitStack() as ctx:
            _body(ctx, tc, nc, t_, use_ln, use_bias)
    nc.compile()
    return nc


def _body(ctx, tc, nc, t_, use_ln, use_bias):
    x, y = t_["x"], t_["y"]

    P = ctx.enter_context(tc.tile_pool(name="persist", bufs=1))

    # ---- first wave of DMAs: what phase A needs immediately -----------------
    ident = P.tile([128, 128], F32R, tag="ident")
    nc.sync.dma_start(ident[:], t_["ident"][:])


    # ---- weight tiles (DMAs emitted later, after the x-transpose wave) ------
    wqk = P.tile([128, 2, E], F32R, tag="wqk")
    wv = P.tile([128, 2, E], F32R, tag="wv")
    wo = P.tile([128, 2, E], F32R, tag="wo")
    w1 = P.tile([128, 4, 2 * E], F32R, tag="w1")
    w2 = P.tile([128, 4, E], F32R, tag="w2")
    ones = P.tile([128, 512], F32R, tag="ones")
    eps = P.tile([128, 1], F32, tag="eps")
    nc.vector.memset(eps[:], 1e-5)
    rows = {}
    row_specs = []
    if use_bias:
        row_specs += [("bqk", E), ("bv", E), ("bo", E), ("b1", 2 * E), ("b2", E)]
    if use_ln:
        row_specs += [("lng", 2 * E), ("lnb", 2 * E)]
    for nm, width in row_specs:
        rows[nm] = P.tile([1, width], F32R, tag=f"r_{nm}", name=f"r_{nm}")

    def load_weights():
        nc.sync.dma_start(wqk[:], t_["Wqk"].rearrange("(c p) e -> p c e", p=128))
        nc.sync.dma_start(ones[:], t_["ones"][:])
        nc.sync.dma_start(wv[:], t_["Wv"].rearrange("(c p) e -> p c e", p=128))
        nc.sync.dma_start(wo[:], t_["Wo"].rearrange("(c p) e -> p c e", p=128))
        nc.sync.dma_start(w1[:], t_["W1"].rearrange("(c p) e -> p c e", p=128))
        nc.sync.dma_start(w2[:], t_["W2"].rearrange("(c p) e -> p c e", p=128))
        for nm, _ in row_specs:
            nc.sync.dma_start(rows[nm][:], t_[nm][:])

    # ---- persistent activations ---------------------------------------------
    xT = [[P.tile([128, N], F32R, tag=f"xT{s}{f}", name=f"xT{s}{f}")
           for f in (0, 1)] for s in (0, 1)]
    mT = [[P.tile([128, T], F32R, tag=f"mT{s}{f}", name=f"mT{s}{f}")
           for f in (0, 1)] for s in (0, 1)]
    mpT = [[P.tile([128, T], F32R, tag=f"mpT{s}{f}", name=f"mpT{s}{f}")
            for f in (0, 1)] for s in (0, 1)]
    st = ctx.enter_context(tc.tile_pool(name="stat", bufs=2))
    sH_es = ExitStack()
    sH0 = sH_es.enter_context(tc.tile_pool(name="sH0", bufs=2))
    qk_es = ExitStack()
    qkp = qk_es.enter_context(tc.tile_pool(name="qkpool", bufs=1))
    qk = [qkp.tile([128, 2 * N], F32R, tag=f"qk{f}", name=f"qk{f}") for f in (0, 1)]
    va_es = [ExitStack(), ExitStack()]
    vap = [va_es[s].enter_context(tc.tile_pool(name=f"vapool{s}", bufs=1))
           for s in (0, 1)]
    va = [vap[s].tile([128, 16, H, DH + 1], F32R, tag=f"va{s}", name=f"va{s}")
          for s in (0, 1)]


    if use_ln:
        gb_bc = []
        with tc.tile_pool(name="pGB", bufs=1, space="PSUM") as pGB:
            for nm in ("lng", "lnb"):
                ps = pGB.tile([128, 2 * E], F32, tag=f"bc_{nm}", name=f"bc_{nm}")
                nc.tensor.matmul(ps[:], lhsT=ones[0:1, 0:128], rhs=rows[nm][:],
                                 start=True, stop=True)
                sb = P.tile([128, 2 * E], F32, tag=f"sb_{nm}", name=f"sb_{nm}")
                nc.vector.tensor_copy(sb[:], ps[:])
                gb_bc.append(sb)
        g_bc, b_bc = gb_bc

    ffn = {}
    projfns = {}

    # --- phase A: transpose x to xT ------------------------------------------
    # --- phase B: projections qk (transposed) and v (normal, augmented) ------
    with tc.tile_pool(name="pA", bufs=3, space="PSUM") as pA, \
         tc.tile_pool(name="xall", bufs=4) as xallp, \
         tc.tile_pool(name="pB", bufs=2, space="PSUM") as pB, \
         tc.tile_pool(name="pBv", bufs=2, space="PSUM") as pBv:
        for s in (0, 1):
            xr = x[s].rearrange("(c p) e -> p c e", p=128)
            for g in range(8):
                xg = xallp.tile([128, 2, E], F32R, tag="xg", name="xg")
                nc.sync.dma_start(xg[:], xr[:, g * 2:(g + 1) * 2, :])
                for ci in (0, 1):
                    c = g * 2 + ci
                    xc = xg[:, ci, :]
                    for f in (0, 1):
                        tp = pA.tile([128, 128], F32R)
                        nc.tensor.transpose(tp[:], xc[:, f * 128:(f + 1) * 128],
                                            ident[:])
                        if f == 0:
                            nc.vector.tensor_copy(
                                xT[s][f][:, c * 128:(c + 1) * 128], tp[:])
                        else:
                            nc.scalar.copy(
                                xT[s][f][:, c * 128:(c + 1) * 128], tp[:])

        load_weights()
        for s in (0, 1):
            nc.vector.tensor_copy(va[s][:, :, :, DH:DH + 1], ones[:, 0:64])

        def qkproj(s, fc, t4, pool, tag):
            ps = pool.tile([128, 512], F32, tag=tag, name="psqk")
            for kc in (0, 1):
                nc.tensor.matmul(
                    ps[:], lhsT=wqk[:, kc, fc * 128:(fc + 1) * 128],
                    rhs=xT[s][kc][:, t4 * 512:(t4 + 1) * 512],
                    start=(kc == 0), stop=use_bias is False and kc == 1)
            if use_bias:
                nc.tensor.matmul(
                    ps[:], lhsT=rows["bqk"][0:1, fc * 128:(fc + 1) * 128],
                    rhs=ones[0:1, 0:512], start=False, stop=True)
            nc.vector.tensor_copy(
                qk[fc][:, s * N + t4 * 512: s * N + (t4 + 1) * 512], ps[:])

        def vproj(s, c, pool, tag):
            ps = pool.tile([128, E], F32, tag=tag, name="psv")
            for kc in (0, 1):
                nc.tensor.matmul(ps[:], lhsT=xT[s][kc][:, c * 128:(c + 1) * 128],
                                 rhs=wv[:, kc, :], start=(kc == 0),
                                 stop=use_bias is False and kc == 1)
            if use_bias:
                nc.tensor.matmul(ps[:], lhsT=ones[0:1, 0:128], rhs=rows["bv"][:],
                                 start=False, stop=True)
            nc.vector.tensor_copy(va[s][:, c, :, 0:DH],
                                  ps[:].rearrange("p (h d) -> p h d", h=H))

        for s_ in (0, 1):
            for t4_ in range(4):
                qkproj(s_, 0, t4_, pB, "psqk")
        for c_ in range(16):
            vproj(1, c_, pBv, "psv")
        projfns["qkproj"] = qkproj
        projfns["vproj"] = vproj

    def attention(s, pS, pAV, pDb, sS, sden, bctag="bc", fc_outer=False,
                  filler=None):
        ko = (1 - s) * N   # keys: other stream
        qo = s * N         # queries: own stream (cols 0:1024 of its region)
        vv = va[1 - s]
        groups = [(qc, fc) for fc in (0, 1) for qc in (0, 1)] if fc_outer \
            else [(qc, fc) for qc in (0, 1) for fc in (0, 1)]
        for qc, fc in groups:
            avp = [pAV.tile([65, 512], F32, tag=f"av{hh}", name=f"av{hh}")
                   for hh in (0, 1)]
            for k2 in range(8):
                sp = [pS.tile([128, 1024], F32, tag="sp", name=f"sp{hh}")
                      for hh in (0, 1)]
                for hh in (0, 1):
                    pr = slice(hh * 64, hh * 64 + 64)
                    for half in (0, 1):
                        kc = k2 * 2 + half
                        nc.tensor.matmul(
                            sp[hh][:, half * 512:(half + 1) * 512],
                            lhsT=qk[fc][pr, ko + kc * 128: ko + (kc + 1) * 128],
                            rhs=qk[fc][pr, qo + qc * 512: qo + (qc + 1) * 512],
                            start=True, stop=True)
                for hh in (0, 1):
                    ss = sS.tile([128, 1024], F32R, tag=f"ss{hh}", name=f"ss{hh}")
                    nc.scalar.activation(ss[:], sp[hh][:], AF.Exp, scale=SCALE)
                    for half in (0, 1):
                        kc = k2 * 2 + half
                        nc.tensor.matmul(
                            avp[hh][:, :],
                            lhsT=vv[:, kc, 2 * fc + hh, :],
                            rhs=ss[:, half * 512:(half + 1) * 512],
                            start=(kc == 0), stop=(kc == 15))
                if filler is not None:
                    next(filler, None)
            den = sden.tile([128, 1024], F32, tag="den", name="den")
            for hh in (0, 1):
                nc.vector.tensor_copy(
                    mT[s][fc][hh * 64:(hh + 1) * 64, qc * 512:(qc + 1) * 512],
                    avp[hh][0:64, :])
                nc.vector.tensor_copy(den[64:65, hh * 512:(hh + 1) * 512],
                                      avp[hh][64:65, :])
            nc.vector.reciprocal(den[64:65, :], den[64:65, :])
            rden = sden.tile([128, 1024], F32R, tag="rden", name="rden")
            nc.vector.tensor_copy(rden[64:65, :], den[64:65, :])
            for hh in (0, 1):
                bc = pDb.tile([128, 512], F32, tag=bctag, name="bc")
                nc.tensor.matmul(bc[:],
                                 lhsT=ones[64:65, 0:128],
                                 rhs=rden[64:65, hh * 512:(hh + 1) * 512],
                                 start=True, stop=True)
                nc.vector.tensor_tensor(
                    mT[s][fc][hh * 64:(hh + 1) * 64, qc * 512:(qc + 1) * 512],
                    mT[s][fc][hh * 64:(hh + 1) * 64, qc * 512:(qc + 1) * 512],
                    bc[hh * 64:(hh + 1) * 64, :], op=OP.mult)

    # ---- per-tile unit emitters for filler interleaving ---------------------
    def mproj_unit(s, e2, qc, pool, tag):
        ps = pool.tile([128, 512], F32, tag=tag, name="de_d")
        for fc in (0, 1):
            nc.tensor.matmul(ps[:], lhsT=wo[:, fc, e2 * 128:(e2 + 1) * 128],
                             rhs=mT[s][fc][:, qc * 512:(qc + 1) * 512],
                             start=(fc == 0), stop=use_bias is False and fc == 1)
        if use_bias:
            nc.tensor.matmul(ps[:], lhsT=rows["bo"][0:1, e2 * 128:(e2 + 1) * 128],
                             rhs=ones[0:1, 0:512], start=False, stop=True)
        nc.vector.tensor_copy(mpT[s][e2][:, qc * 512:(qc + 1) * 512], ps[:])

    def ffn1_init(s, sH):
        h1all = sH.tile([128, 8, 2 * E], BF16, tag=f"h1all{s}", name=f"h1all{s}")
        mvall = st.tile([128, 8, 2], F32, tag=f"mvall{s}", name=f"mvall{s}")
        ffn[s] = [h1all, mvall, None, None, None]

    def ffn1_unit(s, c, pool, tag):
        h1all, mvall = ffn[s][0], ffn[s][1]
        h1 = pool.tile([128, 2 * E], F32, tag=tag, name="de_h")
        lhs_chunks = [xT[s][0][:, c * 128:(c + 1) * 128],
                      xT[s][1][:, c * 128:(c + 1) * 128],
                      mpT[s][0][:, c * 128:(c + 1) * 128],
                      mpT[s][1][:, c * 128:(c + 1) * 128]]
        for j, lt in enumerate(lhs_chunks):
            nc.tensor.matmul(h1[:], lhsT=lt, rhs=w1[:, j, :], start=(j == 0),
                             stop=use_bias is False and j == 3)
        if use_bias:
            nc.tensor.matmul(h1[:], lhsT=ones[0:1, 0:128], rhs=rows["b1"][:],
                             start=False, stop=True)
        s6 = st.tile([128, 6], F32, tag=f"s6_{c % 2}", name="s6")
        nc.vector.bn_stats(s6[:], h1[:])
        nc.vector.bn_aggr(mvall[:, c, :], s6[:])
        nc.vector.tensor_copy(h1all[:, c, :], h1[:])

    def finish_stats(s):
        h1all, mvall = ffn[s][0], ffn[s][1]
        inv = st.tile([128, 8], F32, tag=f"inv{s}", name=f"inv{s}")
        nc.scalar.activation(inv[:], mvall[:, :, 1], AF.Sqrt, bias=eps[:])
        nc.vector.reciprocal(inv[:], inv[:])
        nmi = st.tile([128, 8], F32, tag=f"nmi{s}", name=f"nmi{s}")
        nc.vector.tensor_tensor(nmi[:], mvall[:, :, 0], inv[:], op=OP.mult)
        nc.vector.tensor_scalar_mul(nmi[:], nmi[:], -1.0)
        nmu = None
        if use_ln:
            nmu = st.tile([128, 8], F32, tag=f"nmu{s}", name=f"nmu{s}")
            nc.vector.tensor_scalar_mul(nmu[:], mvall[:, :, 0], -1.0)
        ffn[s][2:] = [inv, nmi, nmu]

    def ffn2_unit(s, c, pE2, pE3, sE, xres):
        h1all, _, inv, nmi, nmu = ffn[s]
        hg = sE.tile([128, 2 * E], F32R, tag="hg", name="hg")
        if not use_ln:
            nc.scalar.activation(hg[:], h1all[:, c, :], AF.Gelu,
                                 bias=nmi[:, c:c + 1], scale=inv[:, c:c + 1])
        else:
            nrm = sE.tile([128, 2 * E], F32, tag="nrm", name="nrm")
            nc.vector.tensor_scalar(nrm[:], h1all[:, c, :], nmu[:, c:c + 1],
                                    inv[:, c:c + 1], op0=OP.add, op1=OP.mult)
            nc.vector.tensor_tensor(nrm[:], nrm[:], g_bc[:], op=OP.mult)
            nc.vector.tensor_tensor(nrm[:], nrm[:], b_bc[:], op=OP.add)
            nc.scalar.activation(hg[:], nrm[:], AF.Gelu)
        tp = pE2.tile([128, 2 * E], F32R, tag="tp", name="tp")
        for j in range(4):
            nc.tensor.transpose(tp[:, j * 128:(j + 1) * 128],
                                hg[:, j * 128:(j + 1) * 128],
                                ident[:])
        hgT = sE.tile([128, 2 * E], F32R, tag="hgT", name="hgT")
        nc.vector.tensor_copy(hgT[:], tp[:])
        yp = pE3.tile([128, E], F32, tag="yp", name="yp")
        for j in range(4):
            nc.tensor.matmul(yp[:], lhsT=hgT[:, j * 128:(j + 1) * 128],
                             rhs=w2[:, j, :], start=(j == 0),
                             stop=use_bias is False and j == 3)
        if use_bias:
            nc.tensor.matmul(yp[:], lhsT=ones[0:1, 0:128], rhs=rows["b2"][:],
                             start=False, stop=True)
        ty = sE.tile([128, E], F32, tag="ty", name="ty")
        nc.vector.tensor_tensor(ty[:], yp[:], xres[s][:, c, :], op=OP.add)
        nc.sync.dma_start(y[s][c * 128:(c + 1) * 128, :], ty[:])

    def make_filler(units):
        for u in units:
            u()
            yield

    # --- C(s0): fc-outer; fillers: qk fc=1 projections, then vproj(0) --------
    with tc.tile_pool(name="pS0", bufs=2, space="PSUM") as pS, \
         tc.tile_pool(name="pAV0", bufs=1, space="PSUM") as pAV, \
         tc.tile_pool(name="pDb0", bufs=2, space="PSUM") as pDb, \
         tc.tile_pool(name="sS0", bufs=2) as sS, \
         tc.tile_pool(name="sden0", bufs=1) as sden:
        qkp_f = projfns["qkproj"]
        vp_f = projfns["vproj"]
        units = [lambda s_=s_, fc_=fc_, t4_=t4_: qkp_f(s_, fc_, t4_, pDb, "bc")
                 for fc_ in (1,) for s_ in (0, 1) for t4_ in range(4)]
        units += [lambda c_=c_: vp_f(0, c_, pDb, "bc") for c_ in range(16)]
        attention(0, pS, pAV, pDb, sS, sden, fc_outer=True,
                  filler=make_filler(units))
    va_es[1].close()  # values of stream 1 no longer needed

    # --- C(s1): qc-outer; fillers: D(s0), FFN1(s0), then D(s1)/FFN1(s1) ------
    with tc.tile_pool(name="pDE0", bufs=2, space="PSUM") as pDE0:
        with tc.tile_pool(name="pS1", bufs=2, space="PSUM") as pS, \
             tc.tile_pool(name="pAV1", bufs=1, space="PSUM") as pAV, \
             tc.tile_pool(name="sS1", bufs=2) as sS, \
             tc.tile_pool(name="sden1", bufs=1) as sden:
            ffn1_init(0, sH0)
            ffn1_init(1, sH0)
            # fillers: stream-0 D/FFN1 anywhere; stream-1 (qc=0) units only
            # from iter 16 on (their mT writes are emitted by end of group 1)
            units = [lambda e2_=e2_, qc_=qc_: mproj_unit(0, e2_, qc_, pDE0, "de")
                     for e2_ in (0, 1) for qc_ in (0, 1)]
            units += [lambda c_=c_: ffn1_unit(0, c_, pDE0, "de") for c_ in range(8)]
            units += [lambda: None] * 4
            units += [lambda e2_=e2_: mproj_unit(1, e2_, 0, pDE0, "de")
                      for e2_ in (0, 1)]
            units += [lambda c_=c_: ffn1_unit(1, c_, pDE0, "de") for c_ in range(4)]
            attention(1, pS, pAV, pDE0, sS, sden, bctag="de",
                      filler=make_filler(units))
        for e2 in (0, 1):
            mproj_unit(1, e2, 1, pDE0, "de")
        for c in range(4, 8):
            ffn1_unit(1, c, pDE0, "de")
        va_es[0].close()
        qk_es.close()

        # --- tail: stats finish + FFN-pass2 ----------------------------------
        with tc.tile_pool(name="pE2", bufs=2, space="PSUM") as pE2, \
             tc.tile_pool(name="pE3", bufs=2, space="PSUM") as pE3, \
             tc.tile_pool(name="sE", bufs=2) as sE, \
             tc.tile_pool(name="xrs", bufs=1) as xrs:
            xres = [xrs.tile([128, 8, E], F32R, tag=f"xres{s}", name=f"xres{s}")
                    for s in (0, 1)]
            for s in (0, 1):
                nc.sync.dma_start(
                    xres[s][:], x[s].rearrange("(c p) e -> p c e", p=128)[:, 0:8, :])
            finish_stats(0)
            for c in range(8):
                ffn2_unit(0, c, pE2, pE3, sE, xres)
            finish_stats(1)
            for c in range(8):
                ffn2_unit(1, c, pE2, pE3, sE, xres)
    sH_es.close()


_PROGRAMS = {}


def get_program(use_ln=False, use_bias=False):
    key = (use_ln, use_bias)
    if key not in _PROGRAMS:
        _PROGRAMS[key] = build_program(use_ln, use_bias)
    return _PROGRAMS[key]


def program_flags(inputs):
    use_ln = not (np.allclose(np.asarray(inputs["ln_g"]), 1.0)
                  and np.allclose(np.asarray(inputs["ln_b"]), 0.0))
    use_bias = any(np.any(np.asarray(inputs[k])) for k in
                   ("bqk", "bv", "bo", "b1", "b2"))
    return use_ln, use_bias


def make_in_maps(inputs, use_ln, use_bias):
    f32 = lambda a: np.ascontiguousarray(np.asarray(a), dtype=np.float32)
    x0, x1 = f32(inputs["x0"]), f32(inputs["x1"])
    shared = {
        "Wqk": f32(inputs["Wqk"]), "Wv": f32(inputs["Wv"]), "Wo": f32(inputs["Wo"]),
        "W1": f32(inputs["W1"]), "W2": f32(inputs["W2"]),
        "ident": np.eye(128, dtype=np.float32),
        "ones_in": np.ones((128, 512), dtype=np.float32),
    }
    if use_bias:
        shared.update({
            "bqk": f32(inputs["bqk"]).reshape(1, E),
            "bv": f32(inputs["bv"]).reshape(1, E),
            "bo": f32(inputs["bo"]).reshape(1, E),
            "b1": f32(inputs["b1"]).reshape(1, 2 * E),
            "b2": f32(inputs["b2"]).reshape(1, E),
        })
    if use_ln:
        shared.update({
            "lng": f32(inputs["ln_g"]).reshape(1, 2 * E),
            "lnb": f32(inputs["ln_b"]).reshape(1, 2 * E),
        })
    in_maps = []
    for core in range(8):
        b, half = core // 2, core % 2

        def roll(xb):
            if half == 0:
                return np.ascontiguousarray(xb)
            return np.ascontiguousarray(np.concatenate([xb[T:], xb[:T]], axis=0))

        im = dict(shared)
        im["x0"] = roll(x0[b])
        im["x1"] = roll(x1[b])
        in_maps.append(im)
    return in_maps


def assemble(results):
    B = 4
    y0 = np.empty((B, N, E), np.float32)
    y1 = np.empty((B, N, E), np.float32)
    for core in range(8):
        b, half = core // 2, core % 2
        y0[b, half * T:(half + 1) * T] = results[core]["y0"]
        y1[b, half * T:(half + 1) * T] = results[core]["y1"]
    return y0, y1




# ---------------------------------------------------------------------------
# Fast execution path: jit once per program, reuse across kernel() calls.
# ---------------------------------------------------------------------------
import jax
from jax.sharding import Mesh, PartitionSpec, NamedSharding
from jax.experimental.shard_map import shard_map

_RUNNERS = {}


def _make_runner(nc, n_cores=8):
    from concourse import bass2jax
    from concourse.bass2jax import _bass_exec_p, install_neuronx_cc_hook
    install_neuronx_cc_hook()
    partition_name = nc.partition_id_tensor.name if nc.partition_id_tensor else None
    in_names, out_names, out_avals, zero_outs = [], [], [], []
    for alloc in nc.m.functions[0].allocations:
        if not isinstance(alloc, mybir.MemoryLocationSet):
            continue
        name = alloc.memorylocations[0].name
        if alloc.kind == "ExternalInput":
            if name != partition_name:
                in_names.append(name)
        elif alloc.kind == "ExternalOutput":
            out_names.append(name)
            out_avals.append(jax.core.ShapedArray(
                tuple(alloc.tensor_shape), mybir.dt.np(alloc.dtype)))
            zero_outs.append(np.zeros(tuple(alloc.tensor_shape),
                                      mybir.dt.np(alloc.dtype)))
    n_params = len(in_names)
    n_outs = len(out_names)
    all_in_names = list(in_names) + list(out_names)
    if partition_name is not None:
        all_in_names.append(partition_name)

    def _body(*args):
        operands = list(args)
        if partition_name is not None:
            operands.append(bass2jax.partition_id_tensor())
        outs = _bass_exec_p.bind(
            *operands,
            out_avals=tuple(out_avals),
            in_names=tuple(all_in_names),
            out_names=tuple(out_names),
            lowering_input_output_aliases=(),
            sim_require_finite=True,
            sim_require_nnan=True,
            nc=nc,
        )
        return tuple(outs)

    devices = jax.devices()[:n_cores]
    mesh = Mesh(np.asarray(devices), ("core",))
    sh = NamedSharding(mesh, PartitionSpec("core"))
    in_specs = (PartitionSpec("core"),) * (n_params + n_outs)
    out_specs = (PartitionSpec("core"),) * n_outs
    donate = tuple(range(n_params, n_params + n_outs))
    sharded = jax.jit(
        shard_map(_body, mesh=mesh, in_specs=in_specs, out_specs=out_specs,
                  check_rep=False),
        donate_argnums=donate, keep_unused=True)

    def run(in_maps):
        concat_in = [
            np.concatenate([np.asarray(in_maps[c][nm]) for c in range(n_cores)],
                           axis=0)
            for nm in in_names
        ]
        dev_in = [jax.device_put(a, sh) for a in concat_in]
        zeros = [jax.device_put(
            np.zeros((n_cores * z.shape[0], *z.shape[1:]), z.dtype), sh)
            for z in zero_outs]
        out_arrs = sharded(*dev_in, *zeros)
        return [
            {nm: np.asarray(out_arrs[i]).reshape(n_cores, *out_avals[i].shape)[c]
             for i, nm in enumerate(out_names)}
            for c in range(n_cores)
        ]

    return run


def get_runner(use_ln=False, use_bias=False):
    key = (use_ln, use_bias)
    if key not in _RUNNERS:
        _RUNNERS[key] = _make_runner(get_program(use_ln, use_bias))
    return _RUNNERS[key]


def kernel(**inputs):
    use_ln, use_bias = program_flags(inputs)
    run = get_runner(use_ln, use_bias)
    in_maps = make_in_maps(inputs, use_ln, use_bias)
    return assemble(run(in_maps))


